# revision 30
# baseline (speedup 1.0000x reference)
"""Trainium2 Bass kernel for nn_AdaptivePhysicsMask.

out[b,i,j] = clip(fixed_bias + alpha * tanh(MLP(feat_i, feat_j)), -10, 10)
fixed_bias = clip(-0.5*relu((e_j-e_i)/1000)*(1-0.3*sigmoid(min(wp_i,wp_j)-5)), -10, 0)

Sharding: 8 NeuronCores, core c owns batch b = c // 4 and i-rows
[(c % 4) * 256, (c % 4 + 1) * 256).  Cores are fully independent (no
collectives); the [2,1024,1024] output is assembled host-side.  All
core-dependence lives in host-side input sharding (one SPMD graph).

Per-core pipeline (1024 j, 256 i):
  prep:  pool wind 4x4 patches (DVE free-dim reduce + PE partition matmul),
         features -> piT/pjT via K=3 matmuls, j-side broadcast tiles.
  main:  for each i-pair t (128):
           h1  = relu(pjT + (pi_t + b1))          DVE tensor_scalar, bf16
           z2  = blockdiag(W2 diag|W3|)^T @ h1    PE -> PSUM f32
           h2  = relu(z2 + |W3| b2)               ACT + DVE split, bf16
           s   = blockdiag(sign W3)^T @ h2        PE -> PSUM [2, 1024]
           T   = tanh(s + b3)                     ACT -> stage bf16
         stage -> dense via SBUF->SBUF DMA (even/odd i split layout:
         dense partition p<64 holds row 2p, p>=64 holds row 2(p-64)+1)
  fixed: dense relu/sigmoid path (same permuted layout),
         out = alpha*T + fixed, un-permuting DMA to DRAM.

Folding: W2' = W2 diag(|W3|), b2' = |W3| b2, sign(W3) in the second matmul
(relu(|w| x) = |w| relu(x)); elev/1000 folded into W1 row 2 (host); the two
reference clips are mathematical no-ops for the attainable value ranges
(fixed_bias in [-0.01, 0], |alpha * tanh| <= alpha).
"""

import numpy as np

import concourse.bass as bass
import concourse.bacc as bacc
import concourse.tile as tile
import concourse.mybir as mybir
from concourse.bass_utils import run_bass_kernel_spmd

F32 = mybir.dt.float32
BF16 = mybir.dt.bfloat16
AF = mybir.ActivationFunctionType
ALU = mybir.AluOpType
NP_BF16 = mybir.dt.np(BF16)

GH = GW = 32
N = GH * GW            # 1024 patches (full j side)
HID = 64
HPIX = WPIX = 128      # wind image pixels
NCORES = 8
NI = 256               # i rows per core
NT = NI // 2           # 128 i-pairs per core
NBLK = 2               # i-blocks of 128 rows each
TBATCH = 16            # i-pairs per stage batch
JC = 512               # matmul free-dim chunk
IPIXH = 32             # pixel rows covering this core's 256 i patches
GYI = IPIXH // 4       # 8 grid rows on the i side


def build_nc(alpha):
    nc = bacc.Bacc("TRN2", target_bir_lowering=False, debug=False,
                   num_devices=NCORES)
    d = {}

    def inp(name, shape, dt=F32):
        d[name] = nc.dram_tensor(name, shape, dt, kind="ExternalInput")

    inp("uw", [HPIX, WPIX])
    inp("vw", [HPIX, WPIX])
    inp("ep", [N])
    inp("uwi", [IPIXH, WPIX])
    inp("vwi", [IPIXH, WPIX])
    inp("epir", [NI])
    inp("epi", [128, NBLK])        # -elev_i, permuted even/odd layout
    inp("w1a", [3, HID])
    inp("w1b", [3, HID])
    inp("w2rep", [128, HID], BF16)
    inp("w3ph", [128, 64 * 64], BF16)
    inp("b1c", [HID, 1])
    inp("b2c", [128, 1])
    inp("b3c", [128, 1])
    inp("pmat", [128, GH])
    inp("pmati", [IPIXH, GYI])
    d["out"] = nc.dram_tensor("out", [NI, N], F32, kind="ExternalOutput")

    _emit(nc, d, alpha)
    return nc, d


def _emit(nc, d, alpha):
    with tile.TileContext(nc) as tc:
        with (
            tc.tile_pool(name="const", bufs=1) as cpool,
            tc.tile_pool(name="prep", bufs=1) as prep,
            tc.tile_pool(name="dram", bufs=1, space="DRAM") as dpool,
            tc.tile_pool(name="h1p", bufs=4) as h1pool,
            tc.tile_pool(name="h2p", bufs=6) as h2pool,
            tc.tile_pool(name="densep", bufs=2) as densep,
            tc.tile_pool(name="fixp", bufs=2) as fixp,
            tc.tile_pool(name="outp", bufs=2) as outp,
        ):
            # ---------------- constants ----------------
            w2rep = cpool.tile([128, HID], BF16)
            w3ph = cpool.tile([128, 64 * 64], BF16)
            w1a = cpool.tile([3, HID], F32)
            w1b = cpool.tile([3, HID], F32)
            b1c = cpool.tile([HID, 1], F32)
            b2c = cpool.tile([128, 1], F32)
            b3c = cpool.tile([128, 1], F32)
            pmat = cpool.tile([128, GH], F32)
            pmati = cpool.tile([IPIXH, GYI], F32)
            epi = cpool.tile([128, NBLK], F32)
            # wind inputs first -- they head the longest prep chain
            uwt = prep.tile([HPIX, WPIX], F32)
            vwt = prep.tile([HPIX, WPIX], F32)
            uwi = prep.tile([IPIXH, WPIX], F32)
            vwi = prep.tile([IPIXH, WPIX], F32)
            nc.sync.dma_start(uwt[:], d["uw"].ap())
            nc.sync.dma_start(vwt[:], d["vw"].ap())
            nc.sync.dma_start(uwi[:], d["uwi"].ap())
            nc.sync.dma_start(vwi[:], d["vwi"].ap())
            for name, t in [("w1a", w1a),
                            ("w1b", w1b), ("b1c", b1c), ("b2c", b2c),
                            ("b3c", b3c), ("pmat", pmat), ("pmati", pmati),
                            ("epi", epi), ("w2rep", w2rep)]:
                nc.sync.dma_start(t[:], d[name].ap())
            # the 1 MB phase-weight table is not needed until the first W3;
            # keep it off the critical HWDGE queue
            nc.gpsimd.dma_start(w3ph[:], d["w3ph"].ap())

            # ---------------- j-side pooling ----------------

            usq = prep.tile([HPIX, WPIX], F32)
            wmag = prep.tile([HPIX, WPIX], F32)
            nc.vector.tensor_mul(usq[:], uwt[:], uwt[:])
            nc.vector.tensor_mul(wmag[:], vwt[:], vwt[:])
            nc.vector.tensor_add(wmag[:], wmag[:], usq[:])
            nc.scalar.activation(wmag[:], wmag[:], AF.Sqrt)

            ppsum_cm = tc.tile_pool(name="ppsum", bufs=1, space="PSUM")
            ppsum = ppsum_cm.__enter__()
            red = prep.tile([HPIX, 3, GH], F32)   # planes: mag, u, v
            for k, src in enumerate((wmag, uwt, vwt)):
                nc.vector.tensor_reduce(
                    red[:, k, :], src[:].rearrange("h (g q) -> h g q", q=4),
                    mybir.AxisListType.X, ALU.add)
            poolps = ppsum.tile([GH, 3, GW], F32, tag="pp")
            for k in range(3):
                nc.tensor.matmul(poolps[:, k, :], pmat[:], red[:, k, :])
            pooled = prep.tile([GH, 3, GW], F32)
            nc.vector.tensor_copy(pooled[:], poolps[:])
            poold = dpool.tile([3, GH, GW], F32)
            nc.sync.dma_start(poold[:].transpose([1, 0, 2]), pooled[:])

            # featT [3, N] rows u_p, v_p, elev ; wm5row [1, N] = wp - 5
            featT = prep.tile([3, N], F32)
            wm5row = prep.tile([1, N], F32)
            pd = poold[:]                        # [3, gy, gx] in DRAM
            nc.sync.dma_start(featT[0:1, :],
                              pd[1].rearrange("gy gx -> (gy gx)").unsqueeze(0))
            nc.sync.dma_start(featT[1:2, :],
                              pd[2].rearrange("gy gx -> (gy gx)").unsqueeze(0))
            nc.sync.dma_start(featT[2:3, :], d["ep"].ap().unsqueeze(0))
            nc.sync.dma_start(wm5row[0:1, :],
                              pd[0].rearrange("gy gx -> (gy gx)").unsqueeze(0))
            nc.vector.tensor_scalar_add(wm5row[:], wm5row[:], -5.0)

            # ---------------- i-side pooling (32-pixel slab) ----------------
            usqi = prep.tile([IPIXH, WPIX], F32)
            wmagi = prep.tile([IPIXH, WPIX], F32)
            nc.vector.tensor_mul(usqi[:], uwi[:], uwi[:])
            nc.vector.tensor_mul(wmagi[:], vwi[:], vwi[:])
            nc.vector.tensor_add(wmagi[:], wmagi[:], usqi[:])
            nc.scalar.activation(wmagi[:], wmagi[:], AF.Sqrt)
            redi = prep.tile([IPIXH, 3, GH], F32)
            for k, src in enumerate((wmagi, uwi, vwi)):
                nc.vector.tensor_reduce(
                    redi[:, k, :], src[:].rearrange("h (g q) -> h g q", q=4),
                    mybir.AxisListType.X, ALU.add)
            pooli = ppsum.tile([GYI, 3, GW], F32, tag="pp")
            for k in range(3):
                nc.tensor.matmul(pooli[:, k, :], pmati[:], redi[:, k, :])
            pooledi = prep.tile([GYI, 3, GW], F32)
            nc.vector.tensor_copy(pooledi[:], pooli[:])
            pooldi = dpool.tile([3, GYI, GW], F32)
            nc.sync.dma_start(pooldi[:].transpose([1, 0, 2]), pooledi[:])

            featTi = prep.tile([3, NI], F32)
            wm5i = prep.tile([1, NI], F32)
            pdi = pooldi[:]
            nc.sync.dma_start(featTi[0:1, :],
                              pdi[1].rearrange("gy gx -> (gy gx)").unsqueeze(0))
            nc.sync.dma_start(featTi[1:2, :],
                              pdi[2].rearrange("gy gx -> (gy gx)").unsqueeze(0))
            nc.sync.dma_start(featTi[2:3, :], d["epir"].ap().unsqueeze(0))
            nc.sync.dma_start(wm5i[0:1, :],
                              pdi[0].rearrange("gy gx -> (gy gx)").unsqueeze(0))
            nc.vector.tensor_scalar_add(wm5i[:], wm5i[:], -5.0)

            # ---------------- pi / pj ----------------
            pj2 = prep.tile([128, N], BF16)
            pjlo = prep.tile([HID, N], BF16)
            for c in range(2):
                pjps = ppsum.tile([HID, JC], F32, tag="pp")
                nc.tensor.matmul(pjps[:], w1b[:],
                                 featT[:, c * JC:(c + 1) * JC])
                nc.scalar.activation(pjlo[:, c * JC:(c + 1) * JC],
                                     pjps[:], AF.Copy)
            nc.sync.dma_start(pj2[0:HID, :], pjlo[:])
            nc.sync.dma_start(pj2[HID:128, :], pjlo[:])

            pips = ppsum.tile([HID, NI], F32, tag="pp")
            piTb = prep.tile([HID, NI], F32)
            nc.tensor.matmul(pips[:], w1a[:], featTi[:])
            nc.scalar.activation(piTb[:], pips[:], AF.Identity,
                                 bias=b1c[:, 0:1])
            # pib2 [128, NT]: col t = [piTb[:,2t] ; piTb[:,2t+1]]
            pib2 = prep.tile([128, NT], F32)
            piview = piTb[:].rearrange("h (t e) -> h t e", e=2)
            nc.sync.dma_start(pib2[0:HID, :], piview[:, :, 0:1].squeeze(2))
            nc.sync.dma_start(pib2[HID:128, :], piview[:, :, 1:2].squeeze(2))

            # ---------------- broadcast + i-side columns ----------------
            elevjB = prep.tile([128, N], F32)
            wpj5B = prep.tile([128, N], F32)
            nc.sync.dma_start(
                elevjB[:], d["ep"].ap().unsqueeze(0).partition_broadcast(128))
            nc.gpsimd.partition_broadcast(wpj5B[:], wm5row[0:1, :])

            # wpi5 [128, NBLK] in the even/odd permuted layout
            wpi5 = prep.tile([128, NBLK], F32)
            wview = wm5i[0, :].rearrange("(b t e) -> t b e", b=NBLK, e=2)
            nc.sync.dma_start(wpi5[0:64, :], wview[:, :, 1:2].squeeze(2))
            nc.sync.dma_start(wpi5[64:128, :], wview[:, :, 0:1].squeeze(2))
            nege = prep.tile([128, NBLK], F32)
            nc.vector.tensor_scalar_mul(nege[:], epi[:], -1.0e-3)

            ppsum_cm.__exit__(None, None, None)

            # ---------------- main loop ----------------
            zpsum_cm = tc.tile_pool(name="zpsum", bufs=3, space="PSUM")
            spsum_cm = tc.tile_pool(name="spsum", bufs=2, space="PSUM")
            zpsum = zpsum_cm.__enter__()
            spsum = spsum_cm.__enter__()
            for blk in range(NBLK):
                er = fixp.tile([128, N], F32, tag="er")
                sg = fixp.tile([128, N], F32, tag="sg")
                Fb = fixp.tile([128, N], F32, tag="Fb")
                nc.scalar.activation(er[:], elevjB[:], AF.Relu,
                                     bias=nege[:, blk:blk + 1], scale=1.0e-3)
                nc.vector.tensor_scalar(sg[:], wpj5B[:],
                                        wpi5[:, blk:blk + 1], None, ALU.min)
                nc.scalar.activation(sg[:], sg[:], AF.Sigmoid)
                nc.vector.tensor_scalar(sg[:], sg[:], 0.15, -0.5,
                                        ALU.mult, ALU.add)
                nc.vector.tensor_mul(Fb[:], er[:], sg[:])

                Tdense = densep.tile([128, N], BF16)
                s2a = spsum.tile([128, JC], F32, tag="s2")
                s2b = spsum.tile([128, JC], F32, tag="s2")
                s2c = [s2a, s2b]
                def emit_w3(h2p_, tlp):
                    # W3 64-phase accumulation: even-i -> rows 64:128,
                    # odd-i -> rows 0:64 of the block-dense psum tile
                    wslp = slice(tlp * 64, tlp * 64 + 64)
                    for c in range(2):
                        sl = slice(c * JC, (c + 1) * JC)
                        nc.tensor.matmul(
                            s2c[c][64:128, :], w3ph[0:64, wslp],
                            h2p_[0:64, sl], start=(tlp == 0),
                            stop=(tlp == 63), tile_position=(0, 64),
                            skip_group_check=True)
                        nc.tensor.matmul(
                            s2c[c][0:64, :], w3ph[64:128, wslp],
                            h2p_[64:128, sl], start=(tlp == 0),
                            stop=(tlp == 63), tile_position=(64, 0),
                            skip_group_check=True)

                def emit_h1(tl_):
                    h1_ = h1pool.tile([128, N], BF16, tag="h1")
                    nc.vector.tensor_scalar(
                        h1_[:], pj2[:], pib2[:, blk * 64 + tl_:blk * 64 + tl_ + 1],
                        0.0, ALU.add, ALU.max)
                    return h1_

                # h1 runs two iterations ahead so the DVE queue never blocks
                # an independent h1 behind a PSUM-waiting relu
                h1q = [emit_h1(0), emit_h1(1), emit_h1(2)]
                pend = None
                for tl in range(64):
                    t = blk * 64 + tl
                    h1 = h1q.pop(0)
                    z2 = zpsum.tile([128, N], F32)
                    for c in range(2):
                        sl = slice(c * JC, (c + 1) * JC)
                        nc.tensor.matmul(
                            z2[0:64, sl], w2rep[0:64, :], h1[0:64, sl],
                            tile_position=(0, 0))
                        nc.tensor.matmul(
                            z2[64:128, sl], w2rep[64:128, :],
                            h1[64:128, sl], tile_position=(64, 64))
                    h2 = h2pool.tile([128, N], BF16)
                    # h2 relu: one full-width op, alternating ACT/DVE 3:1
                    if tl % 4 != 3:
                        nc.scalar.activation(h2[:], z2[:], AF.Relu,
                                             bias=b2c[:, 0:1])
                    else:
                        nc.vector.tensor_scalar(
                            h2[:], z2[:], b2c[:, 0:1], 0.0,
                            ALU.add, ALU.max)
                    # W3 runs one iteration behind so PE never waits on h2
                    if pend is not None:
                        emit_w3(*pend)
                    pend = (h2, tl)
                    if tl + 3 < 64:
                        h1q.append(emit_h1(tl + 3))
                emit_w3(*pend)
                for c in range(2):
                    sl = slice(c * JC, (c + 1) * JC)
                    nc.scalar.activation(Tdense[:, sl], s2c[c][:], AF.Tanh,
                                         bias=b3c[:, 0:1])

                outt = outp.tile([128, N], F32)
                nc.vector.scalar_tensor_tensor(
                    outt[:], Tdense[:], float(alpha), Fb[:],
                    ALU.mult, ALU.add)
                # un-permute: partitions 0:64 hold odd rows, 64:128 even
                r0 = blk * 128
                nc.sync.dma_start(d["out"].ap()[r0 + 1:r0 + 128:2, :],
                                  outt[0:64, :])
                nc.sync.dma_start(d["out"].ap()[r0:r0 + 128:2, :],
                                  outt[64:128, :])
            spsum_cm.__exit__(None, None, None)
            zpsum_cm.__exit__(None, None, None)


def prep_inputs(inputs):
    """Host-side sharding + weight packing -> in_maps (one dict per core)."""
    ep = np.asarray(inputs["elevation_patches"], np.float32)
    u = np.asarray(inputs["u_wind"], np.float32)
    v = np.asarray(inputs["v_wind"], np.float32)
    W1 = np.asarray(inputs["W1"], np.float32)
    b1 = np.asarray(inputs["b1"], np.float32)
    W2 = np.asarray(inputs["W2"], np.float32)
    b2 = np.asarray(inputs["b2"], np.float32)
    W3 = np.asarray(inputs["W3"], np.float32)
    b3 = np.asarray(inputs["b3"], np.float32)

    w3 = W3[:, 0]
    absw3 = np.abs(w3)
    sgnw3 = np.sign(w3).astype(np.float32)
    W2p = (W2 * absw3[None, :]).astype(np.float32)
    b2p = (b2 * absw3).astype(np.float32)
    # w3ph [128, 64*64]: 64 phase matrices [64, 64]; phase p has sgnw3 in
    # column p only.  Rows 0:64 feed the even-i stream, 64:128 the odd-i.
    w3ph = np.zeros((128, 64 * 64), np.float32)
    for p in range(64):
        w3ph[0:HID, p * 64 + p] = sgnw3
        w3ph[HID:128, p * 64 + p] = sgnw3
    W1a = W1[0:3].copy()
    W1b = W1[3:6].copy()
    W1a[2] /= 1000.0
    W1b[2] /= 1000.0

    pmat = np.zeros((128, GH), np.float32)
    for m in range(GH):
        pmat[4 * m:4 * m + 4, m] = 1.0 / 16.0
    pmati = np.ascontiguousarray(pmat[0:IPIXH, 0:GYI])

    common = {
        "w1a": np.ascontiguousarray(W1a),
        "w1b": np.ascontiguousarray(W1b),
        "w2rep": np.concatenate([W2p, W2p], axis=0).astype(NP_BF16),
        "w3ph": w3ph.astype(NP_BF16),
        "b1c": np.ascontiguousarray(b1.reshape(HID, 1)),
        "b2c": np.ascontiguousarray(
            np.concatenate([b2p, b2p]).reshape(128, 1)),
        "b3c": np.full((128, 1), float(b3[0]), np.float32),
        "pmat": pmat,
        "pmati": pmati,
    }

    in_maps = []
    for c in range(NCORES):
        b = c // 4
        i0 = (c % 4) * NI
        py0 = i0 // GW * 4
        eps = ep[b, i0:i0 + NI].reshape(NBLK, 64, 2)
        # dense layout: partitions 0:64 = odd rows (2t+1), 64:128 = even (2t)
        epi = np.concatenate([eps[:, :, 1].T, eps[:, :, 0].T], axis=0)
        m = dict(common)
        m["uw"] = np.ascontiguousarray(u[b])
        m["vw"] = np.ascontiguousarray(v[b])
        m["ep"] = np.ascontiguousarray(ep[b])
        m["uwi"] = np.ascontiguousarray(u[b, py0:py0 + IPIXH, :])
        m["vwi"] = np.ascontiguousarray(v[b, py0:py0 + IPIXH, :])
        m["epir"] = np.ascontiguousarray(ep[b, i0:i0 + NI])
        m["epi"] = np.ascontiguousarray(epi)
        in_maps.append(m)
    return in_maps


def assemble(results):
    out = np.zeros((2, N, N), np.float32)
    for c in range(NCORES):
        b, q = c // 4, c % 4
        out[b, q * NI:(q + 1) * NI, :] = results[c]["out"]
    return out


def kernel(**inputs):
    alpha = float(np.asarray(inputs["alpha"]))
    in_maps = prep_inputs(inputs)
    nc, _ = build_nc(alpha)
    nc.compile()
    res = run_bass_kernel_spmd(nc, in_maps, core_ids=list(range(NCORES)))
    return assemble(res.results)


# revision 31
# speedup vs baseline: 1.0192x; 1.0192x over previous
"""Trainium2 Bass kernel for nn_AdaptivePhysicsMask.

out[b,i,j] = clip(fixed_bias + alpha * tanh(MLP(feat_i, feat_j)), -10, 10)
fixed_bias = clip(-0.5*relu((e_j-e_i)/1000)*(1-0.3*sigmoid(min(wp_i,wp_j)-5)), -10, 0)

Sharding: 8 NeuronCores, core c owns batch b = c // 4 and i-rows
[(c % 4) * 256, (c % 4 + 1) * 256).  Cores are fully independent (no
collectives); the [2,1024,1024] output is assembled host-side.  All
core-dependence lives in host-side input sharding (one SPMD graph).

Per-core pipeline (1024 j, 256 i):
  prep:  pool wind 4x4 patches (DVE free-dim reduce + PE partition matmul),
         features -> piT/pjT via K=3 matmuls, j-side broadcast tiles.
  main:  for each i-pair t (128):
           h1  = relu(pjT + (pi_t + b1))          DVE tensor_scalar, bf16
           z2  = blockdiag(W2 diag|W3|)^T @ h1    PE -> PSUM f32
           h2  = relu(z2 + |W3| b2)               ACT + DVE split, bf16
           s   = blockdiag(sign W3)^T @ h2        PE -> PSUM [2, 1024]
           T   = tanh(s + b3)                     ACT -> stage bf16
         stage -> dense via SBUF->SBUF DMA (even/odd i split layout:
         dense partition p<64 holds row 2p, p>=64 holds row 2(p-64)+1)
  fixed: dense relu/sigmoid path (same permuted layout),
         out = alpha*T + fixed, un-permuting DMA to DRAM.

Folding: W2' = W2 diag(|W3|), b2' = |W3| b2, sign(W3) in the second matmul
(relu(|w| x) = |w| relu(x)); elev/1000 folded into W1 row 2 (host); the two
reference clips are mathematical no-ops for the attainable value ranges
(fixed_bias in [-0.01, 0], |alpha * tanh| <= alpha).
"""

import numpy as np

import concourse.bass as bass
import concourse.bacc as bacc
import concourse.tile as tile
import concourse.mybir as mybir
from concourse.bass_utils import run_bass_kernel_spmd

F32 = mybir.dt.float32
BF16 = mybir.dt.bfloat16
AF = mybir.ActivationFunctionType
ALU = mybir.AluOpType
NP_BF16 = mybir.dt.np(BF16)

GH = GW = 32
N = GH * GW            # 1024 patches (full j side)
HID = 64
HPIX = WPIX = 128      # wind image pixels
NCORES = 8
NI = 256               # i rows per core
NT = NI // 2           # 128 i-pairs per core
NBLK = 2               # i-blocks of 128 rows each
TBATCH = 16            # i-pairs per stage batch
JC = 512               # matmul free-dim chunk
IPIXH = 32             # pixel rows covering this core's 256 i patches
GYI = IPIXH // 4       # 8 grid rows on the i side


def build_nc(alpha):
    nc = bacc.Bacc("TRN2", target_bir_lowering=False, debug=False,
                   num_devices=NCORES)
    d = {}

    def inp(name, shape, dt=F32):
        d[name] = nc.dram_tensor(name, shape, dt, kind="ExternalInput")

    inp("uw", [HPIX, WPIX])
    inp("vw", [HPIX, WPIX])
    inp("ep", [N])
    inp("uwi", [IPIXH, WPIX])
    inp("vwi", [IPIXH, WPIX])
    inp("epir", [NI])
    inp("epi", [128, NBLK])        # -elev_i, permuted even/odd layout
    inp("w1a", [3, HID])
    inp("w1b", [3, HID])
    inp("w2rep", [128, HID], BF16)
    inp("w3ph", [128, 64 * 64], BF16)
    inp("b1c", [HID, 1])
    inp("b2c", [128, 1])
    inp("b3c", [128, 1])
    inp("pmat", [128, GH])
    inp("pmati", [IPIXH, GYI])
    d["out"] = nc.dram_tensor("out", [NI, N], F32, kind="ExternalOutput")

    _emit(nc, d, alpha)
    return nc, d


def _emit(nc, d, alpha):
    with tile.TileContext(nc) as tc:
        with (
            tc.tile_pool(name="const", bufs=1) as cpool,
            tc.tile_pool(name="prep", bufs=1) as prep,
            tc.tile_pool(name="dram", bufs=1, space="DRAM") as dpool,
            tc.tile_pool(name="h1p", bufs=3) as h1pool,
            tc.tile_pool(name="h2p", bufs=4) as h2pool,
            tc.tile_pool(name="densep", bufs=2) as densep,
            tc.tile_pool(name="fixp", bufs=2) as fixp,
            tc.tile_pool(name="outp", bufs=2) as outp,
        ):
            # ---------------- constants ----------------
            w2rep = cpool.tile([128, HID], BF16)
            w3ph = cpool.tile([128, 64 * 64], BF16)
            w1a = cpool.tile([3, HID], F32)
            w1b = cpool.tile([3, HID], F32)
            b1c = cpool.tile([HID, 1], F32)
            b2c = cpool.tile([128, 1], F32)
            b3c = cpool.tile([128, 1], F32)
            pmat = cpool.tile([128, GH], F32)
            pmati = cpool.tile([IPIXH, GYI], F32)
            epi = cpool.tile([128, NBLK], F32)
            # wind inputs first -- they head the longest prep chain
            uwt = prep.tile([HPIX, WPIX], F32)
            vwt = prep.tile([HPIX, WPIX], F32)
            uwi = prep.tile([IPIXH, WPIX], F32)
            vwi = prep.tile([IPIXH, WPIX], F32)
            nc.sync.dma_start(uwt[:], d["uw"].ap())
            nc.sync.dma_start(vwt[:], d["vw"].ap())
            nc.sync.dma_start(uwi[:], d["uwi"].ap())
            nc.sync.dma_start(vwi[:], d["vwi"].ap())
            for name, t in [("w1a", w1a),
                            ("w1b", w1b), ("b1c", b1c), ("b2c", b2c),
                            ("b3c", b3c), ("pmat", pmat), ("pmati", pmati),
                            ("epi", epi), ("w2rep", w2rep)]:
                nc.sync.dma_start(t[:], d[name].ap())
            # the 1 MB phase-weight table is not needed until the first W3;
            # keep it off the critical HWDGE queue
            nc.gpsimd.dma_start(w3ph[:], d["w3ph"].ap())

            # ---------------- j-side pooling ----------------

            usq = prep.tile([HPIX, WPIX], F32)
            wmag = prep.tile([HPIX, WPIX], F32)
            nc.vector.tensor_mul(usq[:], uwt[:], uwt[:])
            nc.vector.tensor_mul(wmag[:], vwt[:], vwt[:])
            nc.vector.tensor_add(wmag[:], wmag[:], usq[:])
            nc.scalar.activation(wmag[:], wmag[:], AF.Sqrt)

            ppsum_cm = tc.tile_pool(name="ppsum", bufs=1, space="PSUM")
            ppsum = ppsum_cm.__enter__()
            red = prep.tile([HPIX, 3, GH], F32)   # planes: mag, u, v
            for k, src in enumerate((wmag, uwt, vwt)):
                nc.vector.tensor_reduce(
                    red[:, k, :], src[:].rearrange("h (g q) -> h g q", q=4),
                    mybir.AxisListType.X, ALU.add)
            poolps = ppsum.tile([GH, 3, GW], F32, tag="pp")
            for k in range(3):
                nc.tensor.matmul(poolps[:, k, :], pmat[:], red[:, k, :])
            pooled = prep.tile([GH, 3, GW], F32)
            nc.vector.tensor_copy(pooled[:], poolps[:])
            poold = dpool.tile([3, GH, GW], F32)
            nc.sync.dma_start(poold[:].transpose([1, 0, 2]), pooled[:])

            # featT [3, N] rows u_p, v_p, elev ; wm5row [1, N] = wp - 5
            featT = prep.tile([3, N], F32)
            wm5row = prep.tile([1, N], F32)
            pd = poold[:]                        # [3, gy, gx] in DRAM
            nc.sync.dma_start(featT[0:1, :],
                              pd[1].rearrange("gy gx -> (gy gx)").unsqueeze(0))
            nc.sync.dma_start(featT[1:2, :],
                              pd[2].rearrange("gy gx -> (gy gx)").unsqueeze(0))
            nc.sync.dma_start(featT[2:3, :], d["ep"].ap().unsqueeze(0))
            nc.sync.dma_start(wm5row[0:1, :],
                              pd[0].rearrange("gy gx -> (gy gx)").unsqueeze(0))
            nc.vector.tensor_scalar_add(wm5row[:], wm5row[:], -5.0)

            # ---------------- i-side pooling (32-pixel slab) ----------------
            usqi = prep.tile([IPIXH, WPIX], F32)
            wmagi = prep.tile([IPIXH, WPIX], F32)
            nc.vector.tensor_mul(usqi[:], uwi[:], uwi[:])
            nc.vector.tensor_mul(wmagi[:], vwi[:], vwi[:])
            nc.vector.tensor_add(wmagi[:], wmagi[:], usqi[:])
            nc.scalar.activation(wmagi[:], wmagi[:], AF.Sqrt)
            redi = prep.tile([IPIXH, 3, GH], F32)
            for k, src in enumerate((wmagi, uwi, vwi)):
                nc.vector.tensor_reduce(
                    redi[:, k, :], src[:].rearrange("h (g q) -> h g q", q=4),
                    mybir.AxisListType.X, ALU.add)
            pooli = ppsum.tile([GYI, 3, GW], F32, tag="pp")
            for k in range(3):
                nc.tensor.matmul(pooli[:, k, :], pmati[:], redi[:, k, :])
            pooledi = prep.tile([GYI, 3, GW], F32)
            nc.vector.tensor_copy(pooledi[:], pooli[:])
            pooldi = dpool.tile([3, GYI, GW], F32)
            nc.sync.dma_start(pooldi[:].transpose([1, 0, 2]), pooledi[:])

            featTi = prep.tile([3, NI], F32)
            wm5i = prep.tile([1, NI], F32)
            pdi = pooldi[:]
            nc.sync.dma_start(featTi[0:1, :],
                              pdi[1].rearrange("gy gx -> (gy gx)").unsqueeze(0))
            nc.sync.dma_start(featTi[1:2, :],
                              pdi[2].rearrange("gy gx -> (gy gx)").unsqueeze(0))
            nc.sync.dma_start(featTi[2:3, :], d["epir"].ap().unsqueeze(0))
            nc.sync.dma_start(wm5i[0:1, :],
                              pdi[0].rearrange("gy gx -> (gy gx)").unsqueeze(0))
            nc.vector.tensor_scalar_add(wm5i[:], wm5i[:], -5.0)

            # ---------------- pi / pj ----------------
            pj2 = prep.tile([128, N], BF16)
            pjlo = prep.tile([HID, N], BF16)
            for c in range(2):
                pjps = ppsum.tile([HID, JC], F32, tag="pp")
                nc.tensor.matmul(pjps[:], w1b[:],
                                 featT[:, c * JC:(c + 1) * JC])
                nc.scalar.activation(pjlo[:, c * JC:(c + 1) * JC],
                                     pjps[:], AF.Copy)
            nc.sync.dma_start(pj2[0:HID, :], pjlo[:])
            nc.sync.dma_start(pj2[HID:128, :], pjlo[:])

            pips = ppsum.tile([HID, NI], F32, tag="pp")
            piTb = prep.tile([HID, NI], F32)
            nc.tensor.matmul(pips[:], w1a[:], featTi[:])
            nc.scalar.activation(piTb[:], pips[:], AF.Identity,
                                 bias=b1c[:, 0:1])
            # pib2 [128, NT]: col t = [piTb[:,2t] ; piTb[:,2t+1]]
            pib2 = prep.tile([128, NT], F32)
            piview = piTb[:].rearrange("h (t e) -> h t e", e=2)
            nc.sync.dma_start(pib2[0:HID, :], piview[:, :, 0:1].squeeze(2))
            nc.sync.dma_start(pib2[HID:128, :], piview[:, :, 1:2].squeeze(2))

            # ---------------- broadcast + i-side columns ----------------
            elevjB = prep.tile([128, N], F32)
            wpj5B = prep.tile([128, N], F32)
            nc.sync.dma_start(
                elevjB[:], d["ep"].ap().unsqueeze(0).partition_broadcast(128))
            nc.gpsimd.partition_broadcast(wpj5B[:], wm5row[0:1, :])

            # wpi5 [128, NBLK] in the even/odd permuted layout
            wpi5 = prep.tile([128, NBLK], F32)
            wview = wm5i[0, :].rearrange("(b t e) -> t b e", b=NBLK, e=2)
            nc.sync.dma_start(wpi5[0:64, :], wview[:, :, 1:2].squeeze(2))
            nc.sync.dma_start(wpi5[64:128, :], wview[:, :, 0:1].squeeze(2))
            nege = prep.tile([128, NBLK], F32)
            nc.vector.tensor_scalar_mul(nege[:], epi[:], -1.0e-3)

            ppsum_cm.__exit__(None, None, None)

            # ---------------- main loop ----------------
            zpsum_cm = tc.tile_pool(name="zpsum", bufs=3, space="PSUM")
            spsum_cm = tc.tile_pool(name="spsum", bufs=2, space="PSUM")
            zpsum = zpsum_cm.__enter__()
            spsum = spsum_cm.__enter__()
            for blk in range(NBLK):
                er = fixp.tile([128, N], F32, tag="er")
                sg = fixp.tile([128, N], F32, tag="sg")
                Fb = fixp.tile([128, N], F32, tag="Fb")
                nc.scalar.activation(er[:], elevjB[:], AF.Relu,
                                     bias=nege[:, blk:blk + 1], scale=1.0e-3)
                nc.vector.tensor_scalar(sg[:], wpj5B[:],
                                        wpi5[:, blk:blk + 1], None, ALU.min)
                nc.scalar.activation(sg[:], sg[:], AF.Sigmoid)
                nc.vector.tensor_scalar(sg[:], sg[:], 0.15, -0.5,
                                        ALU.mult, ALU.add)
                nc.vector.tensor_mul(Fb[:], er[:], sg[:])

                Tdense = densep.tile([128, N], BF16)
                s2a = spsum.tile([128, JC], F32, tag="s2")
                s2b = spsum.tile([128, JC], F32, tag="s2")
                s2c = [s2a, s2b]
                def emit_w3(h2p_, tlp):
                    # W3 64-phase accumulation: even-i -> rows 64:128,
                    # odd-i -> rows 0:64 of the block-dense psum tile
                    wslp = slice(tlp * 64, tlp * 64 + 64)
                    for c in range(2):
                        sl = slice(c * JC, (c + 1) * JC)
                        nc.tensor.matmul(
                            s2c[c][64:128, :], w3ph[0:64, wslp],
                            h2p_[0:64, sl], start=(tlp == 0),
                            stop=(tlp == 63), tile_position=(0, 64),
                            skip_group_check=True)
                        nc.tensor.matmul(
                            s2c[c][0:64, :], w3ph[64:128, wslp],
                            h2p_[64:128, sl], start=(tlp == 0),
                            stop=(tlp == 63), tile_position=(64, 0),
                            skip_group_check=True)

                def emit_h1(tl_):
                    h1_ = h1pool.tile([128, N], BF16, tag="h1")
                    nc.vector.tensor_scalar(
                        h1_[:], pj2[:], pib2[:, blk * 64 + tl_:blk * 64 + tl_ + 1],
                        0.0, ALU.add, ALU.max)
                    return h1_

                # h1 runs two iterations ahead so the DVE queue never blocks
                # an independent h1 behind a PSUM-waiting relu
                h1q = [emit_h1(0), emit_h1(1)]
                pend = None
                for tl in range(64):
                    t = blk * 64 + tl
                    h1 = h1q.pop(0)
                    z2 = zpsum.tile([128, N], F32)
                    for c in range(2):
                        sl = slice(c * JC, (c + 1) * JC)
                        nc.tensor.matmul(
                            z2[0:64, sl], w2rep[0:64, :], h1[0:64, sl],
                            tile_position=(0, 0))
                        nc.tensor.matmul(
                            z2[64:128, sl], w2rep[64:128, :],
                            h1[64:128, sl], tile_position=(64, 64))
                    h2 = h2pool.tile([128, N], BF16)
                    # h2 relu: one full-width op, alternating ACT/DVE 3:1
                    if tl % 4 != 3:
                        nc.scalar.activation(h2[:], z2[:], AF.Relu,
                                             bias=b2c[:, 0:1])
                    else:
                        nc.vector.tensor_scalar(
                            h2[:], z2[:], b2c[:, 0:1], 0.0,
                            ALU.add, ALU.max)
                    # W3 runs one iteration behind so PE never waits on h2
                    if pend is not None:
                        emit_w3(*pend)
                    pend = (h2, tl)
                    if tl + 2 < 64:
                        h1q.append(emit_h1(tl + 2))
                emit_w3(*pend)
                for c in range(2):
                    sl = slice(c * JC, (c + 1) * JC)
                    nc.scalar.activation(Tdense[:, sl], s2c[c][:], AF.Tanh,
                                         bias=b3c[:, 0:1])

                outt = outp.tile([128, N], F32)
                nc.vector.scalar_tensor_tensor(
                    outt[:], Tdense[:], float(alpha), Fb[:],
                    ALU.mult, ALU.add)
                # un-permute: partitions 0:64 hold odd rows, 64:128 even
                r0 = blk * 128
                nc.sync.dma_start(d["out"].ap()[r0 + 1:r0 + 128:2, :],
                                  outt[0:64, :])
                nc.sync.dma_start(d["out"].ap()[r0:r0 + 128:2, :],
                                  outt[64:128, :])
            spsum_cm.__exit__(None, None, None)
            zpsum_cm.__exit__(None, None, None)


def prep_inputs(inputs):
    """Host-side sharding + weight packing -> in_maps (one dict per core)."""
    ep = np.asarray(inputs["elevation_patches"], np.float32)
    u = np.asarray(inputs["u_wind"], np.float32)
    v = np.asarray(inputs["v_wind"], np.float32)
    W1 = np.asarray(inputs["W1"], np.float32)
    b1 = np.asarray(inputs["b1"], np.float32)
    W2 = np.asarray(inputs["W2"], np.float32)
    b2 = np.asarray(inputs["b2"], np.float32)
    W3 = np.asarray(inputs["W3"], np.float32)
    b3 = np.asarray(inputs["b3"], np.float32)

    w3 = W3[:, 0]
    absw3 = np.abs(w3)
    sgnw3 = np.sign(w3).astype(np.float32)
    W2p = (W2 * absw3[None, :]).astype(np.float32)
    b2p = (b2 * absw3).astype(np.float32)
    # w3ph [128, 64*64]: 64 phase matrices [64, 64]; phase p has sgnw3 in
    # column p only.  Rows 0:64 feed the even-i stream, 64:128 the odd-i.
    w3ph = np.zeros((128, 64 * 64), np.float32)
    for p in range(64):
        w3ph[0:HID, p * 64 + p] = sgnw3
        w3ph[HID:128, p * 64 + p] = sgnw3
    W1a = W1[0:3].copy()
    W1b = W1[3:6].copy()
    W1a[2] /= 1000.0
    W1b[2] /= 1000.0

    pmat = np.zeros((128, GH), np.float32)
    for m in range(GH):
        pmat[4 * m:4 * m + 4, m] = 1.0 / 16.0
    pmati = np.ascontiguousarray(pmat[0:IPIXH, 0:GYI])

    common = {
        "w1a": np.ascontiguousarray(W1a),
        "w1b": np.ascontiguousarray(W1b),
        "w2rep": np.concatenate([W2p, W2p], axis=0).astype(NP_BF16),
        "w3ph": w3ph.astype(NP_BF16),
        "b1c": np.ascontiguousarray(b1.reshape(HID, 1)),
        "b2c": np.ascontiguousarray(
            np.concatenate([b2p, b2p]).reshape(128, 1)),
        "b3c": np.full((128, 1), float(b3[0]), np.float32),
        "pmat": pmat,
        "pmati": pmati,
    }

    in_maps = []
    for c in range(NCORES):
        b = c // 4
        i0 = (c % 4) * NI
        py0 = i0 // GW * 4
        eps = ep[b, i0:i0 + NI].reshape(NBLK, 64, 2)
        # dense layout: partitions 0:64 = odd rows (2t+1), 64:128 = even (2t)
        epi = np.concatenate([eps[:, :, 1].T, eps[:, :, 0].T], axis=0)
        m = dict(common)
        m["uw"] = np.ascontiguousarray(u[b])
        m["vw"] = np.ascontiguousarray(v[b])
        m["ep"] = np.ascontiguousarray(ep[b])
        m["uwi"] = np.ascontiguousarray(u[b, py0:py0 + IPIXH, :])
        m["vwi"] = np.ascontiguousarray(v[b, py0:py0 + IPIXH, :])
        m["epir"] = np.ascontiguousarray(ep[b, i0:i0 + NI])
        m["epi"] = np.ascontiguousarray(epi)
        in_maps.append(m)
    return in_maps


def assemble(results):
    out = np.zeros((2, N, N), np.float32)
    for c in range(NCORES):
        b, q = c // 4, c % 4
        out[b, q * NI:(q + 1) * NI, :] = results[c]["out"]
    return out


def kernel(**inputs):
    alpha = float(np.asarray(inputs["alpha"]))
    in_maps = prep_inputs(inputs)
    nc, _ = build_nc(alpha)
    nc.compile()
    res = run_bass_kernel_spmd(nc, in_maps, core_ids=list(range(NCORES)))
    return assemble(res.results)
